# revision 1
# baseline (speedup 1.0000x reference)
"""MoE layer (8 experts, top-2 routing, SwiGLU) on 8 Trainium2 NeuronCores.

Strategy (expert-parallel):
  - Host: run the (tiny) router matmul + softmax + top-2 in numpy, sort the
    (token, slot) pairs by expert id, and build per-expert gathered token
    batches padded to a common capacity.
  - Device (SPMD, core e == expert e): y = (silu(x @ w1e.T) * (x @ w3e.T)) @ w2e.T
    scaled by the per-token gate, all in a feature-major layout so no
    on-chip transposes are needed. Matmuls run in float32r (full fp32
    storage, 1 cycle/row on the PE for moving dim >= 256).
  - Host: un-permute and add the two expert contributions per token.

B, T, C, E, H = 8, 2048, 256, 8, 682; N = B*T = 16384 tokens, top-2.
"""

import os

import numpy as np

import concourse.bass as bass
import concourse.tile as tile
from concourse import bacc, mybir
from concourse.bass_utils import run_bass_kernel_spmd

E = 8
TOP_K = 2
C = 256
H = 682
HP = 768  # H zero-padded to a multiple of 128: partial-row (42-wide)
# fp32 LDWEIGHTS stalls the PE for ~2 matmuls; padding is free PE-wise
# (matmul cost is moving-dim cycles only) and numerically exact (zero
# weights -> silu(0)*0 = 0 contribution).
NTILE = 512  # moving-dim tile (fp32 PSUM bank width)
H_CHUNKS = [(i * 128, 128) for i in range(HP // 128)]
C_CHUNKS = [(i * 128, 128) for i in range(C // 128)]
N_WARMUP_MM = 9  # fp32 dummy matmuls (2 HW mms each, ~450ns cold) covering
# the ~8us startup DMA window to hold the PE HAM warm

_PROGRAM_CACHE: dict[int, object] = {}


def _route(flat: np.ndarray, router_w: np.ndarray):
    """Replicates the reference router: softmax over experts, top-2, renorm."""
    logits = flat @ router_w.T  # [N, E]
    logits -= logits.max(axis=-1, keepdims=True)
    probs = np.exp(logits)
    probs /= probs.sum(axis=-1, keepdims=True)

    n = flat.shape[0]
    ar = np.arange(n)
    i0 = probs.argmax(axis=-1)
    p0 = probs[ar, i0]
    masked = probs.copy()
    masked[ar, i0] = -np.inf
    i1 = masked.argmax(axis=-1)
    p1 = probs[ar, i1]
    denom = p0 + p1 + 1e-9
    return i0, i1, (p0 / denom).astype(np.float32), (p1 / denom).astype(np.float32)


def _n_tiles(cap: int):
    """n-tile (offset, size) list: 512-wide tiles plus one >=256 tail."""
    tiles, off = [], 0
    while cap - off > NTILE:
        tiles.append((off, NTILE))
        off += NTILE
    tiles.append((off, cap - off))
    return tiles


def _build_program(cap: int):
    f32 = mybir.dt.float32
    f32r = mybir.dt.float32r
    ntiles = _n_tiles(cap)
    nt = len(ntiles)

    nc = bacc.Bacc(
        "TRN2",
        target_bir_lowering=False,
        debug=False,
        enable_asserts=False,
        num_devices=E,
    )
    xT_d = nc.dram_tensor("xT", [C, cap], f32r, kind="ExternalInput").ap()
    g_d = nc.dram_tensor("g", [1, cap], f32r, kind="ExternalInput").ap()
    w1T_d = nc.dram_tensor("w1T", [C, H], f32r, kind="ExternalInput").ap()
    w3T_d = nc.dram_tensor("w3T", [C, H], f32r, kind="ExternalInput").ap()
    w2T_d = nc.dram_tensor("w2T", [H, C], f32r, kind="ExternalInput").ap()
    yT_d = nc.dram_tensor("yT", [C, cap], f32, kind="ExternalOutput").ap()

    with tile.TileContext(nc) as tc:
        with (
            tc.tile_pool(name="consts", bufs=1) as consts,
            tc.tile_pool(name="xin", bufs=3) as xin,
            tc.tile_pool(name="gbp", bufs=3) as gbpool,
            tc.tile_pool(name="hbuf", bufs=3) as hbuf,
            tc.tile_pool(name="act", bufs=4) as actp,
            tc.tile_pool(name="yout", bufs=4) as yout,
            tc.tile_pool(name="ps_h", bufs=2, space="PSUM") as ps_h,
            tc.tile_pool(name="ps_y", bufs=3, space="PSUM") as ps_y,
            tc.tile_pool(name="ps_w", bufs=1, space="PSUM") as ps_w,
        ):
            # PE warm-up: dummy matmuls on zeroed SBUF keep the HAM busy
            # (and warm) while the first input DMAs are in flight.
            wz_l = consts.tile([128, 128], f32, tag="wz_l")
            nc.vector.memset(wz_l[:], 0.0)
            for _ in range(N_WARMUP_MM):
                wp = ps_w.tile([128, 128], f32, tag="warm")
                nc.tensor.matmul(wp[:], wz_l[:], wz_l[:], start=True, stop=True)

            x_tiles: dict[int, list] = {}

            def load_x(j):
                no, nsz = ntiles[j]
                ts = []
                for ci, (co, _) in enumerate(C_CHUNKS):
                    xt = xin.tile([128, nsz], f32r, tag=f"x{ci}")
                    nc.sync.dma_start(
                        out=xt[:], in_=xT_d[co : co + 128, no : no + nsz]
                    )
                    ts.append(xt)
                x_tiles[j] = ts

            # Critical-path first: the opening matmul needs x(j0,c0) + w1c0,
            # then x(j0,c1) + w1c1. w3 follows; w2 + gate broadcasts ride the
            # gpsimd queue and are emitted after the first h-phase.
            w1_sb, w3_sb, w2_sb = [], [], []
            load_x(0)
            ncc = len(C_CHUNKS)
            for ci, (co, _) in enumerate(C_CHUNKS):
                t1 = consts.tile([128, HP], f32r, tag=f"w1c{co}")
                nc.sync.dma_start(out=t1[:, :H], in_=w1T_d[co : co + 128, :])
                nc.vector.memset(t1[:, H:].bitcast(f32), 0.0)
                w1_sb.append(t1)
            for co, _ in C_CHUNKS:
                t3 = consts.tile([128, HP], f32r, tag=f"w3c{co}")
                nc.sync.dma_start(out=t3[:, :H], in_=w3T_d[co : co + 128, :])
                nc.vector.memset(t3[:, H:].bitcast(f32), 0.0)
                w3_sb.append(t3)
            load_x(1)

            def emit_h_phase(j):
                """h = silu(x@w1T) * (x@w3T) for n-tile j; returns SBUF tiles."""
                no, nsz = ntiles[j]
                x_sb = x_tiles.pop(j)
                h_tiles = []
                for hi, (ho, hs) in enumerate(H_CHUNKS):
                    h1p = ps_h.tile([hs, nsz], f32, tag="h1")
                    h3p = ps_h.tile([hs, nsz], f32, tag="h3")
                    for ci in range(len(C_CHUNKS)):
                        first = ci == 0
                        last = ci == len(C_CHUNKS) - 1
                        nc.tensor.matmul(
                            h1p[:],
                            w1_sb[ci][:, ho : ho + hs],
                            x_sb[ci][:],
                            start=first,
                            stop=last,
                        )
                        nc.tensor.matmul(
                            h3p[:],
                            w3_sb[ci][:, ho : ho + hs],
                            x_sb[ci][:],
                            start=first,
                            stop=last,
                        )
                    a_sb = actp.tile([hs, nsz], f32r, tag="a")
                    nc.scalar.activation(
                        a_sb[:], h1p[:], mybir.ActivationFunctionType.Silu
                    )
                    h_sb = hbuf.tile([hs, nsz], f32r, tag=f"h{hi}")
                    nc.vector.tensor_mul(h_sb[:], a_sb[:], h3p[:])
                    h_tiles.append(h_sb)
                # gate row broadcast to 128 partitions via stride-0 DMA
                gb_sb = gbpool.tile([128, nsz], f32, tag="gb")
                g_slice = g_d[0:1, no : no + nsz]
                g_bcast = bass.AP(
                    tensor=g_slice.tensor,
                    offset=g_slice.offset,
                    ap=[[0, 128], list(g_slice.ap[-1])],
                )
                nc.gpsimd.dma_start(out=gb_sb[:], in_=g_bcast)
                return h_tiles, gb_sb

            def emit_y_phase(j, h_tiles, gb_sb):
                no, nsz = ntiles[j]
                for ci, (co, _) in enumerate(C_CHUNKS):
                    yp = ps_y.tile([128, nsz], f32, tag="y")
                    for hi, (ho, hs) in enumerate(H_CHUNKS):
                        nc.tensor.matmul(
                            yp[:],
                            w2_sb[hi][:, co : co + 128],
                            h_tiles[hi][:],
                            start=hi == 0,
                            stop=hi == len(H_CHUNKS) - 1,
                        )
                    y_sb = yout.tile([128, nsz], f32, tag="yo")
                    nc.vector.tensor_mul(y_sb[:], yp[:], gb_sb[:])
                    nc.sync.dma_start(
                        out=yT_d[co : co + 128, no : no + nsz], in_=y_sb[:]
                    )

            # Software pipeline: y-phase of tile j is emitted after the
            # h-phase of tile j+1, so the PE never waits on the silu->mul
            # chain at the h->y boundary.
            pending = None
            for j in range(nt):
                if j + 2 < nt:
                    load_x(j + 2)
                hj = emit_h_phase(j)
                if j == 0:
                    for ho, hs in H_CHUNKS:
                        t2 = consts.tile([hs, C], f32r, tag=f"w2h{ho}")
                        real = min(H - ho, hs)
                        if real < hs:
                            nc.vector.memset(t2.bitcast(f32), 0.0)
                        nc.gpsimd.dma_start(out=t2[:real, :], in_=w2T_d[ho : ho + real, :])
                        w2_sb.append(t2)
                if pending is not None:
                    emit_y_phase(*pending)
                pending = (j, *hj)
            emit_y_phase(*pending)

    nc.compile()
    return nc


def _get_program(cap: int):
    if cap not in _PROGRAM_CACHE:
        _PROGRAM_CACHE[cap] = _build_program(cap)
    return _PROGRAM_CACHE[cap]


def kernel(x, router_w, w1, w2, w3, _trace=False):
    B, T, _ = x.shape
    n = B * T
    flat = np.ascontiguousarray(x.reshape(n, C), dtype=np.float32)
    i0, i1, g0, g1 = _route(flat, np.asarray(router_w, dtype=np.float32))

    # Dispatch: for each expert, the token rows routed to it (slot0 then slot1).
    pos = np.empty((2, n), dtype=np.int64)  # row of each (slot, token) in Y
    in_maps = []
    counts = [
        (np.nonzero(i0 == e)[0], np.nonzero(i1 == e)[0]) for e in range(E)
    ]
    cap = max(len(s0) + len(s1) for s0, s1 in counts)
    # round to 128; keep any final partial n-tile >= 256 wide (f32r needs a
    # moving dim >= 256 for the fast PE path)
    cap = max(((cap + 127) // 128) * 128, 256)
    if 0 < cap % NTILE < 256:
        cap = (cap // NTILE) * NTILE + 256

    w1 = np.asarray(w1, dtype=np.float32)
    w2 = np.asarray(w2, dtype=np.float32)
    w3 = np.asarray(w3, dtype=np.float32)
    for e in range(E):
        s0, s1 = counts[e]
        cnt = len(s0) + len(s1)
        base = e * cap
        pos[0, s0] = base + np.arange(len(s0))
        pos[1, s1] = base + len(s0) + np.arange(len(s1))

        xT = np.zeros((C, cap), dtype=np.float32)
        xT[:, : len(s0)] = flat[s0].T
        xT[:, len(s0) : cnt] = flat[s1].T
        g = np.zeros((1, cap), dtype=np.float32)
        g[0, : len(s0)] = g0[s0]
        g[0, len(s0) : cnt] = g1[s1]
        in_maps.append(
            {
                "xT": xT,
                "g": g,
                "w1T": np.ascontiguousarray(w1[e].T),
                "w3T": np.ascontiguousarray(w3[e].T),
                "w2T": np.ascontiguousarray(w2[e].T),
            }
        )

    nc = _get_program(cap)
    if _trace:
        res = run_bass_kernel_spmd(nc, in_maps, list(range(E)), trace=True)
    else:
        # The NTFF trace path needs an antenv.axon_hooks shim this module
        # doesn't install; make sure an ambient BASS_TRACE can't enable it.
        prev = os.environ.get("BASS_NEVER_TRACE")
        os.environ["BASS_NEVER_TRACE"] = "1"
        try:
            res = run_bass_kernel_spmd(nc, in_maps, list(range(E)), trace=False)
        finally:
            if prev is None:
                os.environ.pop("BASS_NEVER_TRACE", None)
            else:
                os.environ["BASS_NEVER_TRACE"] = prev

    Y = np.empty((E * cap, C), dtype=np.float32)
    for e in range(E):
        Y[e * cap : (e + 1) * cap] = res.results[e]["yT"].T
    out = Y[pos[0]] + Y[pos[1]]
    if _trace:
        kernel.last_results = res
    return out.reshape(B, T, C)



# revision 3
# speedup vs baseline: 1.1221x; 1.1221x over previous
"""MoE layer (8 experts, top-2 routing, SwiGLU) on 8 Trainium2 NeuronCores.

Strategy (expert-parallel):
  - Host: run the (tiny) router matmul + softmax + top-2 in numpy, sort the
    (token, slot) pairs by expert id, and build per-expert gathered token
    batches padded to a common capacity.
  - Device (SPMD, core e == expert e): y = (silu(x @ w1e.T) * (x @ w3e.T)) @ w2e.T
    scaled by the per-token gate, all in a feature-major layout so no
    on-chip transposes are needed. Matmuls run in bf16 (fp32 PSUM
    accumulation): 1 cycle/row streaming like fp32r, but LDWEIGHTS gets the
    fast-weight-load path (hidden behind the matmul stream) instead of
    fp32's ~107ns unhidden load, and DMA traffic halves.
  - Host: un-permute and add the two expert contributions per token.

B, T, C, E, H = 8, 2048, 256, 8, 682; N = B*T = 16384 tokens, top-2.
"""

import os

import ml_dtypes
import numpy as np

import concourse.bass as bass
import concourse.tile as tile
from concourse import bacc, mybir
from concourse.bass_utils import run_bass_kernel_spmd

E = 8
TOP_K = 2
C = 256
H = 682
HP = 768  # H zero-padded to a multiple of 128 (zero weights -> silu(0)*0 = 0)
NTILE = 512  # moving-dim tile (fp32 PSUM bank width)
H_CHUNKS = [(i * 128, 128) for i in range(HP // 128)]
C_CHUNKS = [(i * 128, 128) for i in range(C // 128)]
N_WARMUP_MM = 9  # dummy matmuls covering the startup DMA window to hold the
# PE HAM warm

BF16 = ml_dtypes.bfloat16

_PROGRAM_CACHE: dict[int, object] = {}


def _route(flat: np.ndarray, router_w: np.ndarray):
    """Replicates the reference router: softmax over experts, top-2, renorm."""
    logits = flat @ router_w.T  # [N, E]
    logits -= logits.max(axis=-1, keepdims=True)
    probs = np.exp(logits)
    probs /= probs.sum(axis=-1, keepdims=True)

    n = flat.shape[0]
    ar = np.arange(n)
    i0 = probs.argmax(axis=-1)
    p0 = probs[ar, i0]
    masked = probs.copy()
    masked[ar, i0] = -np.inf
    i1 = masked.argmax(axis=-1)
    p1 = probs[ar, i1]
    denom = p0 + p1 + 1e-9
    return i0, i1, (p0 / denom).astype(np.float32), (p1 / denom).astype(np.float32)


def _n_tiles(cap: int):
    """n-tile (offset, size) list: 512-wide tiles plus one >=256 tail."""
    tiles, off = [], 0
    while cap - off > NTILE:
        tiles.append((off, NTILE))
        off += NTILE
    tiles.append((off, cap - off))
    return tiles


def _build_program(cap: int):
    f32 = mybir.dt.float32
    bf16 = mybir.dt.bfloat16
    ntiles = _n_tiles(cap)
    nt = len(ntiles)

    nc = bacc.Bacc(
        "TRN2",
        target_bir_lowering=False,
        debug=False,
        enable_asserts=False,
        num_devices=E,
    )
    xT_d = nc.dram_tensor("xT", [C, cap], bf16, kind="ExternalInput").ap()
    g_d = nc.dram_tensor("g", [1, cap], f32, kind="ExternalInput").ap()
    w1T_d = nc.dram_tensor("w1T", [C, H], bf16, kind="ExternalInput").ap()
    w3T_d = nc.dram_tensor("w3T", [C, H], bf16, kind="ExternalInput").ap()
    w2T_d = nc.dram_tensor("w2T", [H, C], bf16, kind="ExternalInput").ap()
    yT_d = nc.dram_tensor("yT", [C, cap], f32, kind="ExternalOutput").ap()

    with tile.TileContext(nc) as tc:
        with (
            tc.tile_pool(name="consts", bufs=1) as consts,
            tc.tile_pool(name="xin", bufs=3) as xin,
            tc.tile_pool(name="gbp", bufs=3) as gbpool,
            tc.tile_pool(name="hbuf", bufs=3) as hbuf,
            tc.tile_pool(name="act", bufs=4) as actp,
            tc.tile_pool(name="yout", bufs=4) as yout,
            tc.tile_pool(name="ps_h", bufs=2, space="PSUM") as ps_h,
            tc.tile_pool(name="ps_y", bufs=3, space="PSUM") as ps_y,
            tc.tile_pool(name="ps_w", bufs=1, space="PSUM") as ps_w,
        ):
            # PE warm-up: dummy matmuls on zeroed SBUF keep the HAM busy
            # (and warm) while the first input DMAs are in flight.
            wz_l = consts.tile([128, 128], bf16, tag="wz_l")
            nc.vector.memset(wz_l[:], 0.0)
            for _ in range(N_WARMUP_MM):
                wp = ps_w.tile([128, 128], f32, tag="warm")
                nc.tensor.matmul(wp[:], wz_l[:], wz_l[:], start=True, stop=True)

            x_tiles: dict[int, list] = {}

            def load_x(j):
                no, nsz = ntiles[j]
                ts = []
                for ci, (co, _) in enumerate(C_CHUNKS):
                    xt = xin.tile([128, nsz], bf16, tag=f"x{ci}")
                    nc.sync.dma_start(
                        out=xt[:], in_=xT_d[co : co + 128, no : no + nsz]
                    )
                    ts.append(xt)
                x_tiles[j] = ts

            # Critical-path first: the opening matmul needs x(j0,c0) + w1c0,
            # then x(j0,c1) + w1c1. w3 follows; w2 + gate broadcasts ride the
            # gpsimd queue and are emitted after the first h-phase.
            w1_sb, w3_sb, w2_sb = [], [], []
            load_x(0)
            for ci, (co, _) in enumerate(C_CHUNKS):
                t1 = consts.tile([128, HP], bf16, tag=f"w1c{co}")
                nc.sync.dma_start(out=t1[:, :H], in_=w1T_d[co : co + 128, :])
                nc.vector.memset(t1[:, H:], 0.0)
                w1_sb.append(t1)
            for co, _ in C_CHUNKS:
                t3 = consts.tile([128, HP], bf16, tag=f"w3c{co}")
                nc.sync.dma_start(out=t3[:, :H], in_=w3T_d[co : co + 128, :])
                nc.vector.memset(t3[:, H:], 0.0)
                w3_sb.append(t3)
            load_x(1)

            def emit_h_phase(j):
                """h = silu(x@w1T) * (x@w3T) for n-tile j; returns SBUF tiles."""
                no, nsz = ntiles[j]
                x_sb = x_tiles.pop(j)
                h_tiles = []
                ncc = len(C_CHUNKS)
                for hi, (ho, hs) in enumerate(H_CHUNKS):
                    h1p = ps_h.tile([hs, nsz], f32, tag="h1")
                    h3p = ps_h.tile([hs, nsz], f32, tag="h3")
                    # w1 accumulation first so silu can issue while the w3
                    # matmuls stream.
                    for ci in range(ncc):
                        nc.tensor.matmul(
                            h1p[:],
                            w1_sb[ci][:, ho : ho + hs],
                            x_sb[ci][:],
                            start=ci == 0,
                            stop=ci == ncc - 1,
                        )
                    for ci in range(ncc):
                        nc.tensor.matmul(
                            h3p[:],
                            w3_sb[ci][:, ho : ho + hs],
                            x_sb[ci][:],
                            start=ci == 0,
                            stop=ci == ncc - 1,
                        )
                    a_sb = actp.tile([hs, nsz], f32, tag="a")
                    nc.scalar.activation(
                        a_sb[:], h1p[:], mybir.ActivationFunctionType.Silu
                    )
                    h_sb = hbuf.tile([hs, nsz], bf16, tag=f"h{hi}")
                    nc.vector.tensor_mul(h_sb[:], a_sb[:], h3p[:])
                    h_tiles.append(h_sb)
                # gate row broadcast to 128 partitions via stride-0 DMA
                gb_sb = gbpool.tile([128, nsz], f32, tag="gb")
                g_slice = g_d[0:1, no : no + nsz]
                g_bcast = bass.AP(
                    tensor=g_slice.tensor,
                    offset=g_slice.offset,
                    ap=[[0, 128], list(g_slice.ap[-1])],
                )
                nc.gpsimd.dma_start(out=gb_sb[:], in_=g_bcast)
                return h_tiles, gb_sb

            def emit_y_phase(j, h_tiles, gb_sb):
                no, nsz = ntiles[j]
                for ci, (co, _) in enumerate(C_CHUNKS):
                    yp = ps_y.tile([128, nsz], f32, tag="y")
                    for hi, (ho, hs) in enumerate(H_CHUNKS):
                        nc.tensor.matmul(
                            yp[:],
                            w2_sb[hi][:, co : co + 128],
                            h_tiles[hi][:],
                            start=hi == 0,
                            stop=hi == len(H_CHUNKS) - 1,
                        )
                    y_sb = yout.tile([128, nsz], f32, tag="yo")
                    nc.vector.tensor_mul(y_sb[:], yp[:], gb_sb[:])
                    nc.sync.dma_start(
                        out=yT_d[co : co + 128, no : no + nsz], in_=y_sb[:]
                    )

            # Software pipeline: y-phase of tile j is emitted after the
            # h-phase of tile j+1, so the PE never waits on the silu->mul
            # chain at the h->y boundary.
            pending = None
            for j in range(nt):
                if j + 2 < nt:
                    load_x(j + 2)
                hj = emit_h_phase(j)
                if j == 0:
                    for ho, hs in H_CHUNKS:
                        t2 = consts.tile([hs, C], bf16, tag=f"w2h{ho}")
                        real = min(H - ho, hs)
                        if real < hs:
                            nc.vector.memset(t2[:], 0.0)
                        nc.gpsimd.dma_start(out=t2[:real, :], in_=w2T_d[ho : ho + real, :])
                        w2_sb.append(t2)
                if pending is not None:
                    emit_y_phase(*pending)
                pending = (j, *hj)
            emit_y_phase(*pending)

    nc.compile()
    return nc


def _get_program(cap: int):
    if cap not in _PROGRAM_CACHE:
        _PROGRAM_CACHE[cap] = _build_program(cap)
    return _PROGRAM_CACHE[cap]


def kernel(x, router_w, w1, w2, w3, _trace=False):
    B, T, _ = x.shape
    n = B * T
    flat = np.ascontiguousarray(x.reshape(n, C), dtype=np.float32)
    i0, i1, g0, g1 = _route(flat, np.asarray(router_w, dtype=np.float32))

    # Dispatch: for each expert, the token rows routed to it (slot0 then slot1).
    pos = np.empty((2, n), dtype=np.int64)  # row of each (slot, token) in Y
    in_maps = []
    counts = [
        (np.nonzero(i0 == e)[0], np.nonzero(i1 == e)[0]) for e in range(E)
    ]
    cap = max(len(s0) + len(s1) for s0, s1 in counts)
    # round to 128; keep any final partial n-tile >= 256 wide
    cap = max(((cap + 127) // 128) * 128, 256)
    if 0 < cap % NTILE < 256:
        cap = (cap // NTILE) * NTILE + 256

    flat_bf = flat.astype(BF16)
    w1 = np.asarray(w1, dtype=np.float32).astype(BF16)
    w2 = np.asarray(w2, dtype=np.float32).astype(BF16)
    w3 = np.asarray(w3, dtype=np.float32).astype(BF16)
    for e in range(E):
        s0, s1 = counts[e]
        cnt = len(s0) + len(s1)
        base = e * cap
        pos[0, s0] = base + np.arange(len(s0))
        pos[1, s1] = base + len(s0) + np.arange(len(s1))

        xT = np.zeros((C, cap), dtype=BF16)
        xT[:, : len(s0)] = flat_bf[s0].T
        xT[:, len(s0) : cnt] = flat_bf[s1].T
        g = np.zeros((1, cap), dtype=np.float32)
        g[0, : len(s0)] = g0[s0]
        g[0, len(s0) : cnt] = g1[s1]
        in_maps.append(
            {
                "xT": xT,
                "g": g,
                "w1T": np.ascontiguousarray(w1[e].T),
                "w3T": np.ascontiguousarray(w3[e].T),
                "w2T": np.ascontiguousarray(w2[e].T),
            }
        )

    nc = _get_program(cap)
    if _trace:
        res = run_bass_kernel_spmd(nc, in_maps, list(range(E)), trace=True)
    else:
        # The NTFF trace path needs an antenv.axon_hooks shim this module
        # doesn't install; make sure an ambient BASS_TRACE can't enable it.
        prev = os.environ.get("BASS_NEVER_TRACE")
        os.environ["BASS_NEVER_TRACE"] = "1"
        try:
            res = run_bass_kernel_spmd(nc, in_maps, list(range(E)), trace=False)
        finally:
            if prev is None:
                os.environ.pop("BASS_NEVER_TRACE", None)
            else:
                os.environ["BASS_NEVER_TRACE"] = prev

    Y = np.empty((E * cap, C), dtype=np.float32)
    for e in range(E):
        Y[e * cap : (e + 1) * cap] = res.results[e]["yT"].T
    out = Y[pos[0]] + Y[pos[1]]
    if _trace:
        kernel.last_results = res
    return out.reshape(B, T, C)


# revision 9
# speedup vs baseline: 1.1393x; 1.0153x over previous
"""MoE layer (8 experts, top-2 routing, SwiGLU) on 8 Trainium2 NeuronCores.

Strategy (expert-parallel):
  - Host: run the (tiny) router matmul + softmax + top-2 in numpy, sort the
    (token, slot) pairs by expert id, and build per-expert gathered token
    batches padded to a common capacity.
  - Device (SPMD, core e == expert e): y = (silu(x @ w1e.T) * (x @ w3e.T)) @ w2e.T
    scaled by the per-token gate, all in a feature-major layout so no
    on-chip transposes are needed. Matmuls run in bf16 (fp32 PSUM
    accumulation): 1 cycle/row streaming like fp32r, but LDWEIGHTS gets the
    fast-weight-load path (hidden behind the matmul stream) instead of
    fp32's ~107ns unhidden load, and DMA traffic halves.
  - Host: un-permute and add the two expert contributions per token.

B, T, C, E, H = 8, 2048, 256, 8, 682; N = B*T = 16384 tokens, top-2.
"""

import os

import ml_dtypes
import numpy as np

import concourse.bass as bass
import concourse.tile as tile
from concourse import bacc, mybir
from concourse.bass_utils import run_bass_kernel_spmd

E = 8
TOP_K = 2
C = 256
H = 682
HP = 768  # H zero-padded to a multiple of 128 (zero weights -> silu(0)*0 = 0)
NTILE = 512  # moving-dim tile (fp32 PSUM bank width)
H_CHUNKS = [(i * 128, 128) for i in range(HP // 128)]
C_CHUNKS = [(i * 128, 128) for i in range(C // 128)]
N_WARMUP_MM = 5  # dummy matmuls covering the startup DMA window to hold the
# PE HAM warm

BF16 = ml_dtypes.bfloat16

_PROGRAM_CACHE: dict[int, object] = {}


def _route(flat: np.ndarray, router_w: np.ndarray):
    """Replicates the reference router: softmax over experts, top-2, renorm."""
    logits = flat @ router_w.T  # [N, E]
    logits -= logits.max(axis=-1, keepdims=True)
    probs = np.exp(logits)
    probs /= probs.sum(axis=-1, keepdims=True)

    n = flat.shape[0]
    ar = np.arange(n)
    i0 = probs.argmax(axis=-1)
    p0 = probs[ar, i0]
    masked = probs.copy()
    masked[ar, i0] = -np.inf
    i1 = masked.argmax(axis=-1)
    p1 = probs[ar, i1]
    denom = p0 + p1 + 1e-9
    return i0, i1, (p0 / denom).astype(np.float32), (p1 / denom).astype(np.float32)


def _n_tiles(cap: int):
    """n-tile (offset, size) list: 512-wide tiles plus one >=256 tail."""
    tiles, off = [], 0
    while cap - off > NTILE:
        tiles.append((off, NTILE))
        off += NTILE
    tiles.append((off, cap - off))
    return tiles


def _build_program(cap: int):
    f32 = mybir.dt.float32
    bf16 = mybir.dt.bfloat16
    ntiles = _n_tiles(cap)
    nt = len(ntiles)

    nc = bacc.Bacc(
        "TRN2",
        target_bir_lowering=False,
        debug=False,
        enable_asserts=False,
        num_devices=E,
    )
    xT_d = nc.dram_tensor("xT", [C, cap], bf16, kind="ExternalInput").ap()
    g_d = nc.dram_tensor("g", [1, cap], f32, kind="ExternalInput").ap()
    w1T_d = nc.dram_tensor("w1T", [C, H], bf16, kind="ExternalInput").ap()
    w3T_d = nc.dram_tensor("w3T", [C, H], bf16, kind="ExternalInput").ap()
    w2T_d = nc.dram_tensor("w2T", [H, C], bf16, kind="ExternalInput").ap()
    yT_d = nc.dram_tensor("yT", [C, cap], bf16, kind="ExternalOutput").ap()

    with tile.TileContext(nc) as tc:
        with (
            tc.tile_pool(name="consts", bufs=1) as consts,
            tc.tile_pool(name="xin", bufs=3) as xin,
            tc.tile_pool(name="gbp", bufs=3) as gbpool,
            tc.tile_pool(name="hbuf", bufs=3) as hbuf,
            tc.tile_pool(name="act", bufs=4) as actp,
            tc.tile_pool(name="yout", bufs=4) as yout,
            tc.tile_pool(name="ps_h", bufs=2, space="PSUM") as ps_h,
            tc.tile_pool(name="ps_y", bufs=3, space="PSUM") as ps_y,
            tc.tile_pool(name="ps_w", bufs=1, space="PSUM") as ps_w,
        ):
            # PE warm-up: dummy matmuls on zeroed SBUF keep the HAM busy
            # (and warm) while the first input DMAs are in flight.
            wz_l = consts.tile([128, 128], bf16, tag="wz_l")
            nc.vector.memset(wz_l[:], 0.0)
            for _ in range(N_WARMUP_MM):
                wp = ps_w.tile([128, 128], f32, tag="warm")
                nc.tensor.matmul(wp[:], wz_l[:], wz_l[:], start=True, stop=True)

            x_tiles: dict[int, list] = {}

            def load_x(j):
                no, nsz = ntiles[j]
                ts = []
                for ci, (co, _) in enumerate(C_CHUNKS):
                    xt = xin.tile([128, nsz], bf16, tag=f"x{ci}")
                    nc.sync.dma_start(
                        out=xt[:], in_=xT_d[co : co + 128, no : no + nsz]
                    )
                    ts.append(xt)
                x_tiles[j] = ts

            # Critical-path first: the opening matmul needs w1c0 + x(j0,c0),
            # then x(j0,c1) + w1c1 — interleaved on the sync queue so both
            # land ASAP. w3 + w2 + gate broadcasts ride the gpsimd queue in
            # parallel (separate issue engine + DMA channel).
            w1_sb, w3_sb, w2_sb = [], [], []
            w1_t = []
            for ci, (co, _) in enumerate(C_CHUNKS):
                t1 = consts.tile([128, HP], bf16, tag=f"w1c{co}")
                nc.vector.memset(t1[:, H:], 0.0)
                w1_sb.append(t1)
                t3 = consts.tile([128, HP], bf16, tag=f"w3c{co}")
                nc.vector.memset(t3[:, H:], 0.0)
                w3_sb.append(t3)
            nc.sync.dma_start(out=w1_sb[0][:, :H], in_=w1T_d[0:128, :])
            nsz0 = ntiles[0][1]
            for ci, (co, _) in enumerate(C_CHUNKS):
                xt = xin.tile([128, nsz0], bf16, tag=f"x{ci}")
                nc.sync.dma_start(out=xt[:], in_=xT_d[co : co + 128, 0:nsz0])
                if ci == 0:
                    nc.sync.dma_start(out=w1_sb[1][:, :H], in_=w1T_d[128:256, :])
                x_tiles.setdefault(0, []).append(xt)
            for ci, (co, _) in enumerate(C_CHUNKS):
                nc.gpsimd.dma_start(
                    out=w3_sb[ci][:, :H], in_=w3T_d[co : co + 128, :]
                )
            for ho, hs in H_CHUNKS:
                t2 = consts.tile([hs, C], bf16, tag=f"w2h{ho}")
                real = min(H - ho, hs)
                if real < hs:
                    nc.vector.memset(t2[:], 0.0)
                nc.gpsimd.dma_start(out=t2[:real, :], in_=w2T_d[ho : ho + real, :])
                w2_sb.append(t2)
            load_x(1)

            def emit_h_phase(j):
                """h = silu(x@w1T) * (x@w3T) for n-tile j; returns SBUF tiles."""
                no, nsz = ntiles[j]
                x_sb = x_tiles.pop(j)
                h_tiles = []
                ncc = len(C_CHUNKS)
                for hi, (ho, hs) in enumerate(H_CHUNKS):
                    h1p = ps_h.tile([hs, nsz], f32, tag="h1")
                    h3p = ps_h.tile([hs, nsz], f32, tag="h3")
                    # w1 accumulation first so silu can issue while the w3
                    # matmuls stream.
                    for ci in range(ncc):
                        nc.tensor.matmul(
                            h1p[:],
                            w1_sb[ci][:, ho : ho + hs],
                            x_sb[ci][:],
                            start=ci == 0,
                            stop=ci == ncc - 1,
                        )
                    for ci in range(ncc):
                        nc.tensor.matmul(
                            h3p[:],
                            w3_sb[ci][:, ho : ho + hs],
                            x_sb[ci][:],
                            start=ci == 0,
                            stop=ci == ncc - 1,
                        )
                    a_sb = actp.tile([hs, nsz], f32, tag="a")
                    nc.scalar.activation(
                        a_sb[:], h1p[:], mybir.ActivationFunctionType.Silu
                    )
                    h_sb = hbuf.tile([hs, nsz], bf16, tag=f"h{hi}")
                    nc.vector.tensor_mul(h_sb[:], a_sb[:], h3p[:])
                    h_tiles.append(h_sb)
                # gate row broadcast to 128 partitions via stride-0 DMA
                gb_sb = gbpool.tile([128, nsz], f32, tag="gb")
                g_slice = g_d[0:1, no : no + nsz]
                g_bcast = bass.AP(
                    tensor=g_slice.tensor,
                    offset=g_slice.offset,
                    ap=[[0, 128], list(g_slice.ap[-1])],
                )
                nc.gpsimd.dma_start(out=gb_sb[:], in_=g_bcast)
                return h_tiles, gb_sb

            def emit_y_phase(j, h_tiles, gb_sb):
                no, nsz = ntiles[j]
                for ci, (co, _) in enumerate(C_CHUNKS):
                    yp = ps_y.tile([128, nsz], f32, tag="y")
                    for hi, (ho, hs) in enumerate(H_CHUNKS):
                        nc.tensor.matmul(
                            yp[:],
                            w2_sb[hi][:, co : co + 128],
                            h_tiles[hi][:],
                            start=hi == 0,
                            stop=hi == len(H_CHUNKS) - 1,
                        )
                    y_sb = yout.tile([128, nsz], bf16, tag="yo")
                    nc.vector.tensor_mul(y_sb[:], yp[:], gb_sb[:])
                    nc.sync.dma_start(
                        out=yT_d[co : co + 128, no : no + nsz], in_=y_sb[:]
                    )

            # Software pipeline: y-phase of tile j is emitted after the
            # h-phase of tile j+1, so the PE never waits on the silu->mul
            # chain at the h->y boundary.
            pending = None
            for j in range(nt):
                if j + 2 < nt:
                    load_x(j + 2)
                hj = emit_h_phase(j)
                if pending is not None:
                    emit_y_phase(*pending)
                pending = (j, *hj)
            emit_y_phase(*pending)

    nc.compile()
    return nc


def _get_program(cap: int):
    if cap not in _PROGRAM_CACHE:
        _PROGRAM_CACHE[cap] = _build_program(cap)
    return _PROGRAM_CACHE[cap]


def kernel(x, router_w, w1, w2, w3, _trace=False):
    B, T, _ = x.shape
    n = B * T
    flat = np.ascontiguousarray(x.reshape(n, C), dtype=np.float32)
    i0, i1, g0, g1 = _route(flat, np.asarray(router_w, dtype=np.float32))

    # Dispatch: for each expert, the token rows routed to it (slot0 then slot1).
    pos = np.empty((2, n), dtype=np.int64)  # row of each (slot, token) in Y
    in_maps = []
    counts = [
        (np.nonzero(i0 == e)[0], np.nonzero(i1 == e)[0]) for e in range(E)
    ]
    cap = max(len(s0) + len(s1) for s0, s1 in counts)
    # round to 128; keep any final partial n-tile >= 256 wide
    cap = max(((cap + 127) // 128) * 128, 256)
    if 0 < cap % NTILE < 256:
        cap = (cap // NTILE) * NTILE + 256

    flat_bf = flat.astype(BF16)
    w1 = np.asarray(w1, dtype=np.float32).astype(BF16)
    w2 = np.asarray(w2, dtype=np.float32).astype(BF16)
    w3 = np.asarray(w3, dtype=np.float32).astype(BF16)
    for e in range(E):
        s0, s1 = counts[e]
        cnt = len(s0) + len(s1)
        base = e * cap
        pos[0, s0] = base + np.arange(len(s0))
        pos[1, s1] = base + len(s0) + np.arange(len(s1))

        xT = np.zeros((C, cap), dtype=BF16)
        xT[:, : len(s0)] = flat_bf[s0].T
        xT[:, len(s0) : cnt] = flat_bf[s1].T
        g = np.zeros((1, cap), dtype=np.float32)
        g[0, : len(s0)] = g0[s0]
        g[0, len(s0) : cnt] = g1[s1]
        in_maps.append(
            {
                "xT": xT,
                "g": g,
                "w1T": np.ascontiguousarray(w1[e].T),
                "w3T": np.ascontiguousarray(w3[e].T),
                "w2T": np.ascontiguousarray(w2[e].T),
            }
        )

    nc = _get_program(cap)
    if _trace:
        res = run_bass_kernel_spmd(nc, in_maps, list(range(E)), trace=True)
    else:
        # The NTFF trace path needs an antenv.axon_hooks shim this module
        # doesn't install; make sure an ambient BASS_TRACE can't enable it.
        prev = os.environ.get("BASS_NEVER_TRACE")
        os.environ["BASS_NEVER_TRACE"] = "1"
        try:
            res = run_bass_kernel_spmd(nc, in_maps, list(range(E)), trace=False)
        finally:
            if prev is None:
                os.environ.pop("BASS_NEVER_TRACE", None)
            else:
                os.environ["BASS_NEVER_TRACE"] = prev

    Y = np.empty((E * cap, C), dtype=np.float32)
    for e in range(E):
        Y[e * cap : (e + 1) * cap] = res.results[e]["yT"].T
    out = Y[pos[0]] + Y[pos[1]]
    if _trace:
        kernel.last_results = res
    return out.reshape(B, T, C)
